# revision 49
# baseline (speedup 1.0000x reference)
"""Bidirectional GQA attention block (B=4,T=2048,C=2048,H=16,KVH=4) on 8 TRN2 cores.

Sharding: data-parallel over (batch, seq-half): core c handles batch b=c//2,
query tokens [r0, r0+1024) with r0=(c%2)*1024.  k/v are computed for the full
batch sequence on each core (2x duplicated k/v-proj work, ~8% overhead) so no
cross-core communication is needed; the final output is a pure concatenation.

v2 pipeline (everything staged in fp16; all matmuls fp16 at full PE rate;
PSUM accumulation fp32):
  P1a: q^T = wq^T x^T channel-major -> qTs (DRAM, fp16).  Sum-of-squares for
       RMSNorm via (1/C)-ones matmul; rs = 1/sqrt(mean+eps) (ACT sqrt + DVE
       recip), folded into per-token rope tables (q tables pre-scaled by
       1/sqrt(head_dim) on host).
  P1b: k^T and v projections written DIRECTLY into SBUF-resident tiles
       (no DRAM round trip).  Rope k-tables scaled per 512-token chunk.
  P2:  per kv-group g: kR = kT*c2k + kTswap*s2k (swap via SBUF->SBUF DMA);
       per head: qR likewise (q re-read from DRAM).  logits^T = kR_chunk qR
       per 128-key chunk, exp on ACT -> S fp16.  Softmax denominator via a
       4-level DVE pairwise tree (fp16, 2x mode) + ONE 512-row ones-matmul
       per block (16x less PE than the naive ones-matmul).  den rows for the
       8 blocks of a group batch into one [8,512] PSUM tile -> ONE DVE
       reciprocal per group.  y^T = v S accumulated in PSUM, staged to SBUF
       (ACT), divided by den (DVE) into the SBUF-resident yT tile.
  P3:  out = yT.T wo with PSUM accumulation over the 16 head-chunks.
"""
import sys
import os

sys.path.insert(0, "/opt/trn_rl_repo")

import numpy as np

B, T, C = 4, 2048, 2048
N_HEAD, N_KV_HEAD = 16, 4
HEAD_DIM = C // N_HEAD  # 128
KV_DIM = N_KV_HEAD * HEAD_DIM  # 512
EPS = 1e-5
TQ = 1024  # query tokens per core
N_CORES = 8

_CACHE = {}


def _build_nc(reps=1, trace_sim=False):
    import concourse.bass as bass
    import concourse.mybir as mybir
    import concourse.tile as tile
    from concourse import bacc

    F32 = mybir.dt.float32
    F16 = mybir.dt.float16
    AF = mybir.ActivationFunctionType

    nc = bacc.Bacc("TRN2", target_bir_lowering=False, debug=False)

    def ein(name, shape, dt=F16):
        return nc.dram_tensor(name, shape, dt, kind="ExternalInput").ap()

    xT = ein("xT", [C, T])          # x[b].T  (c_in, tok) fp16
    xTq = ein("xTq", [C, TQ])       # x[b].T[:, r0:r0+TQ] fp16
    wq = ein("wq", [C, C])
    wk = ein("wk", [C, KV_DIM])
    wv = ein("wv", [C, KV_DIM])
    wo = ein("wo", [C, C])
    c2q = ein("c2q", [128, TQ])     # [cos;cos] / sqrt(HEAD_DIM), q token slice
    s2q = ein("s2q", [128, TQ])     # [sin;-sin] / sqrt(HEAD_DIM)
    c2k = ein("c2k", [128, T])      # [cos;cos] (unscaled)
    s2k = ein("s2k", [128, T])
    qnw = ein("qnw", [128, 16], F32)  # q_norm_w.reshape(16,128).T
    knw = ein("knw", [128, 4], F32)
    out = nc.dram_tensor("out", [TQ, C], F32, kind="ExternalOutput").ap()

    ones_d = nc.inline_tensor(np.ones((128, 1), np.float16), name="onesc").ap()
    onesq_d = nc.inline_tensor(
        np.full((128, 1), 1.0 / C, np.float16), name="onesqc"
    ).ap()
    onesk_d = nc.inline_tensor(
        np.full((128, 1), 1.0 / KV_DIM, np.float16), name="oneskc"
    ).ap()
    eps_d = nc.inline_tensor(np.full((1, 1), EPS, np.float32), name="epsc").ap()

    # DRAM scratch: only q^T is staged (k/v/y live in SBUF)
    qTs = nc.dram_tensor("qTs", [C, TQ], F16).ap()

    def r3(ap, p=128):
        # (c*p, n) -> (c, p, n)
        return ap.rearrange("(c p) n -> c p n", p=p)

    def rp(ap, p=128):
        # (c*p, n) -> (p, c, n)
        return ap.rearrange("(c p) n -> p c n", p=p)

    # state shared between phases of one rep
    st = {}

    def p1a(tc, cs):
        """q^T projection + rmsnorm stats; loads rope tables; allocates
        SBUF-resident k/v/y tiles.  DMA issue order is latency-ordered:
        the first matmul's operands (xq0, wql0) go first."""
        pp, pps = st["pp"], st["pps"]
        with tc.tile_pool(name="p1q", bufs=1) as p1, \
             tc.tile_pool(name="wqlp", bufs=3) as pw, \
             tc.tile_pool(name="ev1", bufs=3) as pe, \
             tc.tile_pool(name="tmp1", bufs=3) as pt, \
             tc.tile_pool(name="rsp", bufs=2) as prs:
            xqs, wqls = [], []
            xq = p1.tile([128, 16, 512], F16, name="xq0", tag="xq0")
            nc.sync.dma_start(xq[:], rp(xTq)[:, :, 0:512])
            xqs.append(xq)
            wql = pw.tile([128, 16, 128], F16, name="wql", tag="wql")
            nc.sync.dma_start(wql[:], rp(wq)[:, :, 0:128])
            wqls.append(wql)
            xq = p1.tile([128, 16, 512], F16, name="xq1", tag="xq1")
            nc.sync.dma_start(xq[:], rp(xTq)[:, :, 512:1024])
            xqs.append(xq)
            for cout in (1, 2):
                wql = pw.tile([128, 16, 128], F16, name="wql", tag="wql")
                nc.sync.dma_start(
                    wql[:], rp(wq)[:, :, cout * 128:(cout + 1) * 128]
                )
                wqls.append(wql)
            # rope tables (scaled at end of P1a / in P1b)
            for nm, src in (("c2qs", c2q), ("s2qs", s2q),
                            ("c2ks", c2k), ("s2ks", s2k)):
                t = st["ptab"].tile([128, src.shape[-1]], F16, name=nm)
                nc.sync.dma_start(t[:], src)
                st[nm] = t
            # prefetch P1b weights so k/v proj starts without a DMA stall
            wkt = st["pwkv"].tile([128, 16, KV_DIM], F16, name="wkt")
            nc.sync.dma_start(wkt[:], rp(wk))
            st["wkt"] = wkt
            wvt = st["pwkv"].tile([128, 16, KV_DIM], F16, name="wvt")
            nc.sync.dma_start(wvt[:], rp(wv))
            st["wvt"] = wvt
            # SBUF-resident k/v/y + o-proj partial sums (yc 0..7 half)
            st["kT"] = st["pres"].tile([128, 4, T], F16, name="kT_res")
            st["v"] = st["pres"].tile([128, 16, KV_DIM], F16, name="v_res")
            st["yT"] = st["pres"].tile([128, 16, TQ], F16, name="yT_res")
            st["part"] = st["pres"].tile([128, 32, 512], F16, name="part_res")

            ssq_ps = [
                pps.tile([1, 512], F32, name=f"ssqq{tq}", tag=f"ssqq{tq}",
                         bufs=1)
                for tq in range(2)
            ]
            for cout in range(16):
                if cout < 3:
                    wql = wqls[cout]
                else:
                    # ACT's DMA queue: these block on buffer recycling, and
                    # on the in-order SP queue they would head-of-line block
                    # the P1b prefetches behind them
                    wql = pw.tile([128, 16, 128], F16, name="wql", tag="wql")
                    nc.scalar.dma_start(
                        wql[:], rp(wq)[:, :, cout * 128:(cout + 1) * 128]
                    )
                for tq in range(2):
                    ps = pp.tile([128, 512], F32, name="psq", tag="ps")
                    for kc in range(16):
                        nc.tensor.matmul(
                            ps[:], wql[:, kc, :], xqs[tq][:, kc, :],
                            start=(kc == 0), stop=(kc == 15),
                        )
                    qsb = pe.tile([128, 512], F16, name="qsb", tag="qsb")
                    nc.scalar.activation(
                        qsb[:], ps[:], AF.Copy, scale=cs["qnw"][:, cout:cout + 1]
                    )
                    # stores ride the gpsimd queue so they never block loads
                    nc.gpsimd.dma_start(
                        r3(qTs)[cout, :, tq * 512:(tq + 1) * 512], qsb[:]
                    )
                    sq = pt.tile([128, 512], F16, name="sqq", tag="sq")
                    nc.scalar.activation(sq[:], ps[:], AF.Square)
                    nc.tensor.matmul(
                        ssq_ps[tq][:], cs["onesq"][:], sq[:],
                        start=(cout == 0), stop=(cout == 15),
                    )
            for tq in range(2):
                sl = slice(tq * 512, (tq + 1) * 512)
                sd = prs.tile([1, 512], F32, name="sdq", tag="sdq")
                nc.scalar.activation(sd[:], ssq_ps[tq][:], AF.Sqrt,
                                     bias=cs["eps"][:])
                rs = prs.tile([1, 512], F32, name="rsq", tag="rsq")
                nc.vector.reciprocal(rs[:], sd[:])
                bcq = prs.tile([128, 512], F32, name="bcq", tag="bcq")
                nc.gpsimd.partition_broadcast(bcq[:], rs[:])
                nc.vector.tensor_mul(st["c2qs"][:, sl], st["c2qs"][:, sl], bcq[:])
                nc.vector.tensor_mul(st["s2qs"][:, sl], st["s2qs"][:, sl], bcq[:])

    def p1b(tc, cs):
        """k^T and v projections into SBUF-resident tiles + k-table scaling."""
        kT_res, v_res = st["kT"], st["v"]
        wkt, wvt = st["wkt"], st["wvt"]
        pp, pps = st["pp"], st["pps"]
        with tc.tile_pool(name="xsp", bufs=2) as pxs, \
             tc.tile_pool(name="tmp2", bufs=3) as pt, \
             tc.tile_pool(name="rsk", bufs=2) as prs:
            for tk in range(4):
                tsl = slice(tk * 512, (tk + 1) * 512)
                xs = pxs.tile([128, 16, 512], F16, name="xsc", tag="xsc")
                nc.sync.dma_start(xs[:], rp(xT)[:, :, tsl])
                ssqk_ps = pps.tile([1, 512], F32, name="ssqk", tag="ssqk",
                                   bufs=2)
                for co in range(4):
                    ps = pp.tile([128, 512], F32, name="psk", tag="ps")
                    for kc in range(16):
                        nc.tensor.matmul(
                            ps[:], wkt[:, kc, co * 128:(co + 1) * 128], xs[:, kc, :],
                            start=(kc == 0), stop=(kc == 15),
                        )
                    nc.scalar.activation(
                        kT_res[:, co, tsl], ps[:], AF.Copy,
                        scale=cs["knw"][:, co:co + 1]
                    )
                    sq = pt.tile([128, 512], F16, name="sqk", tag="sq")
                    nc.scalar.activation(sq[:], ps[:], AF.Square)
                    nc.tensor.matmul(
                        ssqk_ps[:], cs["onesk"][:], sq[:],
                        start=(co == 0), stop=(co == 3),
                    )
                sd = prs.tile([1, 512], F32, name="sdk", tag="sdk")
                nc.scalar.activation(sd[:], ssqk_ps[:], AF.Sqrt, bias=cs["eps"][:])
                rs = prs.tile([1, 512], F32, name="rsk", tag="rsk")
                nc.vector.reciprocal(rs[:], sd[:])
                bck = prs.tile([128, 512], F32, name="bck", tag="bck")
                nc.gpsimd.partition_broadcast(bck[:], rs[:])
                nc.vector.tensor_mul(st["c2ks"][:, tsl], st["c2ks"][:, tsl], bck[:])
                nc.vector.tensor_mul(st["s2ks"][:, tsl], st["s2ks"][:, tsl], bck[:])
                for vt in range(4):
                    ps = pp.tile([128, 512], F32, name="psv", tag="ps")
                    for kc in range(16):
                        nc.tensor.matmul(
                            ps[:], xs[:, kc, vt * 128:(vt + 1) * 128], wvt[:, kc, :],
                            start=(kc == 0), stop=(kc == 15),
                        )
                    nc.scalar.activation(v_res[:, tk * 4 + vt, :], ps[:], AF.Copy)

    def p2_head(tc, cs, pools, g, hh, qR, kR, filler=None):
        """One head: both q-chunks' S matmuls + exps first (one long PE
        stretch; exp of chunk 0 overlaps S matmuls of chunk 1), then the
        DVE den trees, then yt/den matmuls and the division.  The den
        matmul sits AFTER yt so the in-order PE never waits on the tree.
        `filler` emits independent PE work (o-proj partials) between the
        S matmuls and the exp-dependent yt matmuls."""
        pS, p8, p4, p2t, p1t, prc, pbc, ppS, ppd, ppy = pools
        h = g * 4 + hh
        g128 = slice(g * 128, (g + 1) * 128)
        S_sbs, t1s = [], []
        for qc in range(2):
            qsl = slice(qc * 512, (qc + 1) * 512)
            S_sb = pS.tile([128, 16, 512], F16, name="S_sb", tag="S")
            for j in range(8):
                sps = ppS.tile([128, 2, 512], F32, name="sps", tag="sps")
                for i in range(2):
                    kc = 2 * j + i
                    nc.tensor.matmul(
                        sps[:, i, :], kR[:, kc * 128:(kc + 1) * 128], qR[:, qsl],
                        start=True, stop=True,
                    )
                nc.scalar.activation(S_sb[:, 2 * j:2 * j + 2, :], sps[:], AF.Exp)
            S_sbs.append(S_sb)
        if filler is not None:
            filler()
        for qc in range(2):
            # denominator: 4-level pairwise tree on DVE (fp16, 2x mode)
            S_sb = S_sbs[qc]
            t8 = p8.tile([128, 8, 512], F16, name="t8", tag="t8")
            nc.vector.tensor_add(t8[:], S_sb[:, 0:8, :], S_sb[:, 8:16, :])
            t4 = p4.tile([128, 4, 512], F16, name="t4", tag="t4")
            nc.vector.tensor_add(t4[:], t8[:, 0:4, :], t8[:, 4:8, :])
            t2 = p2t.tile([128, 2, 512], F16, name="t2", tag="t2")
            nc.vector.tensor_add(t2[:], t4[:, 0:2, :], t4[:, 2:4, :])
            t1 = p1t.tile([128, 512], F16, name="t1", tag="t1")
            nc.vector.tensor_add(t1[:], t2[:, 0, :], t2[:, 1, :])
            t1s.append(t1)
        for qc in range(2):
            qsl = slice(qc * 512, (qc + 1) * 512)
            yt_ps = ppy.tile([128, 512], F32, name="ytp", tag="ytp")
            for kc in range(16):
                nc.tensor.matmul(
                    yt_ps[:], st["v"][:, kc, g128], S_sbs[qc][:, kc, :],
                    start=(kc == 0), stop=(kc == 15),
                )
            den_ps = ppd.tile([1, 512], F32, name="den", tag="den")
            nc.tensor.matmul(den_ps[:], cs["ones"][:], t1s[qc][:],
                             start=True, stop=True)
            rcp = prc.tile([1, 512], F16, name="rcp", tag="rcp")
            with nc.allow_low_precision(reason="softmax denom fits fp16"):
                nc.vector.reciprocal(rcp[:], den_ps[:])
            bcr = pbc.tile([128, 512], F16, name="bcr", tag="bcr")
            nc.gpsimd.partition_broadcast(bcr[:], rcp[:])
            nc.vector.tensor_mul(st["yT"][:, h, qsl], yt_ps[:], bcr[:])

    def p2(tc, cs):
        """attention over 4 kv-groups x 4 heads x 2 q-chunks."""
        kT_res, yT_res = st["kT"], st["yT"]
        c2qs, s2qs, c2ks, s2ks = st["c2qs"], st["s2qs"], st["c2ks"], st["s2ks"]
        with tc.tile_pool(name="ksw", bufs=2) as pks, \
             tc.tile_pool(name="krp", bufs=2) as pkr, \
             tc.tile_pool(name="qh", bufs=2) as pqh, \
             tc.tile_pool(name="Sp", bufs=2) as pS, \
             tc.tile_pool(name="tr8", bufs=1) as p8, \
             tc.tile_pool(name="tr4", bufs=1) as p4, \
             tc.tile_pool(name="tr2", bufs=1) as p2t, \
             tc.tile_pool(name="tr1", bufs=2) as p1t, \
             tc.tile_pool(name="rcb", bufs=2) as prc, \
             tc.tile_pool(name="bcb", bufs=2) as pbc, \
             tc.tile_pool(name="woh", bufs=2) as pwh, \
             tc.tile_pool(name="sps", bufs=2, space="PSUM") as ppS, \
             tc.tile_pool(name="dnp", bufs=1, space="PSUM") as ppd, \
             tc.tile_pool(name="p3a", bufs=1, space="PSUM") as pp3a, \
             tc.tile_pool(name="ytp", bufs=2, space="PSUM") as ppy:
            pools = (pS, p8, p4, p2t, p1t, prc, pbc, ppS, ppd, ppy)
            woch = [None]

            def filler(h):
                # o-proj partial tiles (yc 0..7) as PE filler while the
                # ACT exp stream catches up.  4 of the 32 (co,qt) tiles
                # per head over heads 8..15.
                def emit():
                    co = (h - 8) // 2
                    if h % 2 == 0 and h < 16:
                        w = pwh.tile([128, 8, 512], F16, name="woch", tag="woch")
                        nc.sync.dma_start(
                            w[:], rp(wo)[:, 0:8, co * 512:(co + 1) * 512]
                        )
                        woch[0] = w
                    for i in range(4):
                        idx = (h - 8) * 4 + i
                        co, qt = idx // 8, idx % 8
                        ps3 = pp3a.tile([128, 512], F32, name="ps3", tag="p3")
                        for yc in range(8):
                            nc.tensor.matmul(
                                ps3[:],
                                st["yT"][:, yc, qt * 128:(qt + 1) * 128],
                                woch[0][:, yc, :],
                                start=(yc == 0), stop=(yc == 7),
                            )
                        nc.scalar.activation(st["part"][:, idx, :], ps3[:],
                                             AF.Copy)
                return emit

            def rope_q(h):
                # per-head rope chain; emitted one head AHEAD of its use so
                # the in-order DVE queue has qR ready before the S matmuls
                qTt = pqh.tile([128, TQ], F16, name="qTt", tag="qTt")
                nc.gpsimd.dma_start(qTt[:], r3(qTs)[h])
                qSw = pqh.tile([128, TQ], F16, name="qSw", tag="qSw")
                nc.gpsimd.dma_start(qSw[0:64, :], r3(qTs)[h, 64:128, :])
                nc.gpsimd.dma_start(qSw[64:128, :], r3(qTs)[h, 0:64, :])
                qA = pqh.tile([128, TQ], F16, name="qA", tag="qA", bufs=1)
                nc.vector.tensor_mul(qA[:], qTt[:], c2qs[:])
                nc.vector.tensor_mul(qSw[:], qSw[:], s2qs[:])
                qR = pqh.tile([128, TQ], F16, name="qR", tag="qR")
                nc.vector.tensor_add(qR[:], qA[:], qSw[:])
                return qR

            qR_cur = rope_q(0)
            for g in range(N_KV_HEAD):
                kSw = pks.tile([128, T], F16, name="kSw", tag="kSw", bufs=1)
                nc.gpsimd.dma_start(kSw[0:64, :], kT_res[64:128, g, :])
                nc.gpsimd.dma_start(kSw[64:128, :], kT_res[0:64, g, :])
                kA = pkr.tile([128, T], F16, name="kA", tag="kA", bufs=1)
                nc.vector.tensor_mul(kA[:], kT_res[:, g, :], c2ks[:])
                nc.vector.tensor_mul(kSw[:], kSw[:], s2ks[:])
                kR = pkr.tile([128, T], F16, name="kR", tag="kR")
                nc.vector.tensor_add(kR[:], kA[:], kSw[:])
                for hh in range(4):
                    h = g * 4 + hh
                    qR_next = rope_q(h + 1) if h + 1 < N_HEAD else None
                    p2_head(tc, cs, pools, g, hh, qR_cur, kR,
                            filler=filler(h) if h >= 8 else None)
                    qR_cur = qR_next

    def p3(tc, cs):
        """output projection finish: yc 8..15 half + staged yc 0..7
        partials (computed during P2)."""
        yT_res = st["yT"]
        with tc.tile_pool(name="woc", bufs=2) as pwo, \
             tc.tile_pool(name="ev3", bufs=4) as pe3, \
             tc.tile_pool(name="pp3", bufs=4, space="PSUM") as pp3:
            for co in range(4):
                woc = pwo.tile([128, 8, 512], F16, name="woc", tag="woc")
                nc.sync.dma_start(
                    woc[:], rp(wo)[:, 8:16, co * 512:(co + 1) * 512]
                )
                for qt in range(8):
                    idx = co * 8 + qt
                    ps = pp3.tile([128, 512], F32, name="pso", tag="ps")
                    for yc in range(8):
                        nc.tensor.matmul(
                            ps[:], yT_res[:, 8 + yc, qt * 128:(qt + 1) * 128],
                            woc[:, yc, :],
                            start=(yc == 0), stop=(yc == 7),
                        )
                    osb = pe3.tile([128, 512], F32, name="osb", tag="osb")
                    nc.vector.tensor_add(osb[:], ps[:], st["part"][:, idx, :])
                    nc.gpsimd.dma_start(
                        out[qt * 128:(qt + 1) * 128, co * 512:(co + 1) * 512],
                        osb[:],
                    )

    with tile.TileContext(nc, trace_sim=trace_sim) as tc:
        with tc.tile_pool(name="const", bufs=1) as cpool:
            cs = {}
            for nm, src in (("ones", ones_d), ("onesq", onesq_d),
                            ("onesk", onesk_d)):
                t = cpool.tile([128, 1], F16, name=nm + "_t")
                nc.sync.dma_start(t[:], src)
                cs[nm] = t
            cs["eps"] = cpool.tile([1, 1], F32, name="eps_t")
            nc.sync.dma_start(cs["eps"][:], eps_d)
            cs["qnw"] = cpool.tile([128, 16], F32, name="qnw_t")
            nc.sync.dma_start(cs["qnw"][:], qnw)
            cs["knw"] = cpool.tile([128, 4], F32, name="knw_t")
            nc.sync.dma_start(cs["knw"][:], knw)

            for rep in range(reps):
                with tc.tile_pool(name="resid", bufs=1) as pres, \
                     tc.tile_pool(name="tabs", bufs=1) as ptab:
                    st.clear()
                    st["pres"], st["ptab"] = pres, ptab
                    with tc.tile_pool(name="wkv", bufs=1) as pwkv, \
                         tc.tile_pool(name="pp1", bufs=4, space="PSUM") as pp, \
                         tc.tile_pool(name="ssqp", bufs=1, space="PSUM") as pps:
                        st["pwkv"] = pwkv
                        st["pp"], st["pps"] = pp, pps
                        p1a(tc, cs)
                        p1b(tc, cs)
                    p2(tc, cs)
                    p3(tc, cs)

    nc.compile()
    return nc


def _make_in_maps(inputs):
    F16 = np.float16
    x = np.asarray(inputs["x"], np.float32)
    cos = np.asarray(inputs["cos"], np.float32)
    sin = np.asarray(inputs["sin"], np.float32)
    wq = np.ascontiguousarray(np.asarray(inputs["wq"], np.float32).astype(F16))
    wk = np.ascontiguousarray(np.asarray(inputs["wk"], np.float32).astype(F16))
    wv = np.ascontiguousarray(np.asarray(inputs["wv"], np.float32).astype(F16))
    wo = np.ascontiguousarray(np.asarray(inputs["wo"], np.float32).astype(F16))
    qnw = np.ascontiguousarray(
        np.asarray(inputs["q_norm_w"], np.float32).reshape(16, 128).T
    )
    knw = np.ascontiguousarray(
        np.asarray(inputs["k_norm_w"], np.float32).reshape(4, 128).T
    )

    cf = cos[0, :, 0, :].T  # (64, T)
    sf = sin[0, :, 0, :].T
    c2k = np.concatenate([cf, cf], 0)  # (128, T)
    s2k = np.concatenate([sf, -sf], 0)
    scale = 1.0 / np.sqrt(np.float32(HEAD_DIM))
    c2k16 = np.ascontiguousarray(c2k.astype(F16))
    s2k16 = np.ascontiguousarray(s2k.astype(F16))

    in_maps = []
    for c in range(N_CORES):
        b, r0 = c // 2, (c % 2) * TQ
        xT = np.ascontiguousarray(x[b].T.astype(F16))
        in_maps.append({
            "xT": xT,
            "xTq": np.ascontiguousarray(xT[:, r0:r0 + TQ]),
            "wq": wq, "wk": wk, "wv": wv, "wo": wo,
            "c2q": np.ascontiguousarray((c2k[:, r0:r0 + TQ] * scale).astype(F16)),
            "s2q": np.ascontiguousarray((s2k[:, r0:r0 + TQ] * scale).astype(F16)),
            "c2k": c2k16, "s2k": s2k16,
            "qnw": qnw, "knw": knw,
        })
    return in_maps


def run(inputs, **spmd_kwargs):
    from concourse import bass_utils

    if "nc" not in _CACHE:
        _CACHE["nc"] = _build_nc()
    nc = _CACHE["nc"]
    res = bass_utils.run_bass_kernel_spmd(
        nc, _make_in_maps(inputs), core_ids=list(range(N_CORES)), **spmd_kwargs
    )
    out = np.empty((B, T, C), np.float32)
    for c in range(N_CORES):
        b, r0 = c // 2, (c % 2) * TQ
        out[b, r0:r0 + TQ, :] = res.results[c]["out"]
    return out, res


def kernel(**inputs):
    out, _ = run(inputs)
    return out


# revision 52
# speedup vs baseline: 1.0047x; 1.0047x over previous
"""Bidirectional GQA attention block (B=4,T=2048,C=2048,H=16,KVH=4) on 8 TRN2 cores.

Sharding: data-parallel over (batch, seq-half): core c handles batch b=c//2,
query tokens [r0, r0+1024) with r0=(c%2)*1024.  k/v are computed for the full
batch sequence on each core (2x duplicated k/v-proj work, ~8% overhead) so no
cross-core communication is needed; the final output is a pure concatenation.

v2 pipeline (everything staged in fp16; all matmuls fp16 at full PE rate;
PSUM accumulation fp32):
  P1a: q^T = wq^T x^T channel-major -> qTs (DRAM, fp16).  Sum-of-squares for
       RMSNorm via (1/C)-ones matmul; rs = 1/sqrt(mean+eps) (ACT sqrt + DVE
       recip), folded into per-token rope tables (q tables pre-scaled by
       1/sqrt(head_dim) on host).
  P1b: k^T and v projections written DIRECTLY into SBUF-resident tiles
       (no DRAM round trip).  Rope k-tables scaled per 512-token chunk.
  P2:  per kv-group g: kR = kT*c2k + kTswap*s2k (swap via SBUF->SBUF DMA);
       per head: qR likewise (q re-read from DRAM).  logits^T = kR_chunk qR
       per 128-key chunk, exp on ACT -> S fp16.  Softmax denominator via a
       4-level DVE pairwise tree (fp16, 2x mode) + ONE 512-row ones-matmul
       per block (16x less PE than the naive ones-matmul).  den rows for the
       8 blocks of a group batch into one [8,512] PSUM tile -> ONE DVE
       reciprocal per group.  y^T = v S accumulated in PSUM, staged to SBUF
       (ACT), divided by den (DVE) into the SBUF-resident yT tile.
  P3:  out = yT.T wo with PSUM accumulation over the 16 head-chunks.
"""
import sys
import os

sys.path.insert(0, "/opt/trn_rl_repo")

import numpy as np

B, T, C = 4, 2048, 2048
N_HEAD, N_KV_HEAD = 16, 4
HEAD_DIM = C // N_HEAD  # 128
KV_DIM = N_KV_HEAD * HEAD_DIM  # 512
EPS = 1e-5
TQ = 1024  # query tokens per core
N_CORES = 8

_CACHE = {}


def _build_nc(reps=1, trace_sim=False):
    import concourse.bass as bass
    import concourse.mybir as mybir
    import concourse.tile as tile
    from concourse import bacc

    F32 = mybir.dt.float32
    F16 = mybir.dt.float16
    AF = mybir.ActivationFunctionType

    nc = bacc.Bacc("TRN2", target_bir_lowering=False, debug=False)

    def ein(name, shape, dt=F16):
        return nc.dram_tensor(name, shape, dt, kind="ExternalInput").ap()

    xT = ein("xT", [C, T])          # x[b].T  (c_in, tok) fp16
    xTq = ein("xTq", [C, TQ])       # x[b].T[:, r0:r0+TQ] fp16
    wq = ein("wq", [C, C])
    wk = ein("wk", [C, KV_DIM])
    wv = ein("wv", [C, KV_DIM])
    wo = ein("wo", [C, C])
    c2q = ein("c2q", [128, TQ])     # [cos;cos] / sqrt(HEAD_DIM), q token slice
    s2q = ein("s2q", [128, TQ])     # [sin;-sin] / sqrt(HEAD_DIM)
    c2k = ein("c2k", [128, T])      # [cos;cos] (unscaled)
    s2k = ein("s2k", [128, T])
    qnw = ein("qnw", [128, 16], F32)  # q_norm_w.reshape(16,128).T
    knw = ein("knw", [128, 4], F32)
    out = nc.dram_tensor("out", [TQ, C], F32, kind="ExternalOutput").ap()

    ones_d = nc.inline_tensor(np.ones((128, 1), np.float16), name="onesc").ap()
    onesq_d = nc.inline_tensor(
        np.full((128, 1), 1.0 / C, np.float16), name="onesqc"
    ).ap()
    onesk_d = nc.inline_tensor(
        np.full((128, 1), 1.0 / KV_DIM, np.float16), name="oneskc"
    ).ap()
    eps_d = nc.inline_tensor(np.full((1, 1), EPS, np.float32), name="epsc").ap()

    # DRAM scratch: only q^T is staged (k/v/y live in SBUF)
    qTs = nc.dram_tensor("qTs", [C, TQ], F16).ap()

    def r3(ap, p=128):
        # (c*p, n) -> (c, p, n)
        return ap.rearrange("(c p) n -> c p n", p=p)

    def rp(ap, p=128):
        # (c*p, n) -> (p, c, n)
        return ap.rearrange("(c p) n -> p c n", p=p)

    # state shared between phases of one rep
    st = {}

    def p1a(tc, cs):
        """q^T projection + rmsnorm stats; loads rope tables; allocates
        SBUF-resident k/v/y tiles.  DMA issue order is latency-ordered:
        the first matmul's operands (xq0, wql0) go first."""
        pp, pps = st["pp"], st["pps"]
        with tc.tile_pool(name="p1q", bufs=1) as p1, \
             tc.tile_pool(name="wqlp", bufs=3) as pw, \
             tc.tile_pool(name="ev1", bufs=3) as pe, \
             tc.tile_pool(name="tmp1", bufs=3) as pt, \
             tc.tile_pool(name="rsp", bufs=2) as prs:
            xqs, wqls = [], []
            xq = p1.tile([128, 16, 512], F16, name="xq0", tag="xq0")
            nc.sync.dma_start(xq[:], rp(xTq)[:, :, 0:512])
            xqs.append(xq)
            wql = pw.tile([128, 16, 128], F16, name="wql", tag="wql")
            nc.sync.dma_start(wql[:], rp(wq)[:, :, 0:128])
            wqls.append(wql)
            xq = p1.tile([128, 16, 512], F16, name="xq1", tag="xq1")
            nc.sync.dma_start(xq[:], rp(xTq)[:, :, 512:1024])
            xqs.append(xq)
            for cout in (1, 2):
                wql = pw.tile([128, 16, 128], F16, name="wql", tag="wql")
                nc.sync.dma_start(
                    wql[:], rp(wq)[:, :, cout * 128:(cout + 1) * 128]
                )
                wqls.append(wql)
            # rope tables (scaled at end of P1a / in P1b)
            for nm, src in (("c2qs", c2q), ("s2qs", s2q),
                            ("c2ks", c2k), ("s2ks", s2k)):
                t = st["ptab"].tile([128, src.shape[-1]], F16, name=nm)
                nc.sync.dma_start(t[:], src)
                st[nm] = t
            # prefetch P1b weights so k/v proj starts without a DMA stall
            wkt = st["pwkv"].tile([128, 16, KV_DIM], F16, name="wkt")
            nc.sync.dma_start(wkt[:], rp(wk))
            st["wkt"] = wkt
            wvt = st["pwkv"].tile([128, 16, KV_DIM], F16, name="wvt")
            nc.sync.dma_start(wvt[:], rp(wv))
            st["wvt"] = wvt
            # SBUF-resident k/v/y + o-proj partial sums (yc 0..7 half)
            st["kT"] = st["pres"].tile([128, 4, T], F16, name="kT_res")
            st["v"] = st["pres"].tile([128, 16, KV_DIM], F16, name="v_res")
            st["yT"] = st["pres"].tile([128, 16, TQ], F16, name="yT_res")
            st["part"] = st["pres"].tile([128, 32, 512], F16, name="part_res")

            ssq_ps = [
                pps.tile([1, 512], F32, name=f"ssqq{tq}", tag=f"ssqq{tq}",
                         bufs=1)
                for tq in range(2)
            ]
            for cout in range(16):
                if cout < 3:
                    wql = wqls[cout]
                else:
                    # ACT's DMA queue: these wait on buffer recycling and on
                    # the in-order SP queue they head-of-line block the P1b
                    # xs0 prefetch emitted after them (SP can only bypass 4
                    # waiting instructions)
                    wql = pw.tile([128, 16, 128], F16, name="wql", tag="wql")
                    nc.scalar.dma_start(
                        wql[:], rp(wq)[:, :, cout * 128:(cout + 1) * 128]
                    )
                for tq in range(2):
                    ps = pp.tile([128, 512], F32, name="psq", tag="ps")
                    for kc in range(16):
                        nc.tensor.matmul(
                            ps[:], wql[:, kc, :], xqs[tq][:, kc, :],
                            start=(kc == 0), stop=(kc == 15),
                        )
                    qsb = pe.tile([128, 512], F16, name="qsb", tag="qsb")
                    nc.scalar.activation(
                        qsb[:], ps[:], AF.Copy, scale=cs["qnw"][:, cout:cout + 1]
                    )
                    # stores ride the gpsimd queue so they never block loads
                    nc.gpsimd.dma_start(
                        r3(qTs)[cout, :, tq * 512:(tq + 1) * 512], qsb[:]
                    )
                    sq = pt.tile([128, 512], F16, name="sqq", tag="sq")
                    nc.scalar.activation(sq[:], ps[:], AF.Square)
                    nc.tensor.matmul(
                        ssq_ps[tq][:], cs["onesq"][:], sq[:],
                        start=(cout == 0), stop=(cout == 15),
                    )
            for tq in range(2):
                sl = slice(tq * 512, (tq + 1) * 512)
                sd = prs.tile([1, 512], F32, name="sdq", tag="sdq")
                nc.scalar.activation(sd[:], ssq_ps[tq][:], AF.Sqrt,
                                     bias=cs["eps"][:])
                rs = prs.tile([1, 512], F32, name="rsq", tag="rsq")
                nc.vector.reciprocal(rs[:], sd[:])
                bcq = prs.tile([128, 512], F32, name="bcq", tag="bcq")
                nc.gpsimd.partition_broadcast(bcq[:], rs[:])
                nc.vector.tensor_mul(st["c2qs"][:, sl], st["c2qs"][:, sl], bcq[:])
                nc.vector.tensor_mul(st["s2qs"][:, sl], st["s2qs"][:, sl], bcq[:])

    def p1b(tc, cs):
        """k^T and v projections into SBUF-resident tiles + k-table scaling."""
        kT_res, v_res = st["kT"], st["v"]
        wkt, wvt = st["wkt"], st["wvt"]
        pp, pps = st["pp"], st["pps"]
        with tc.tile_pool(name="xsp", bufs=2) as pxs, \
             tc.tile_pool(name="tmp2", bufs=3) as pt, \
             tc.tile_pool(name="rsk", bufs=2) as prs:
            for tk in range(4):
                tsl = slice(tk * 512, (tk + 1) * 512)
                xs = pxs.tile([128, 16, 512], F16, name="xsc", tag="xsc")
                # tk>=2 waits on the xs ring; keep those off the SP queue so
                # P2's kSw/qTt prefetches behind them issue early
                eng = nc.sync if tk < 2 else nc.scalar
                eng.dma_start(xs[:], rp(xT)[:, :, tsl])
                ssqk_ps = pps.tile([1, 512], F32, name="ssqk", tag="ssqk",
                                   bufs=2)
                for co in range(4):
                    ps = pp.tile([128, 512], F32, name="psk", tag="ps")
                    for kc in range(16):
                        nc.tensor.matmul(
                            ps[:], wkt[:, kc, co * 128:(co + 1) * 128], xs[:, kc, :],
                            start=(kc == 0), stop=(kc == 15),
                        )
                    nc.scalar.activation(
                        kT_res[:, co, tsl], ps[:], AF.Copy,
                        scale=cs["knw"][:, co:co + 1]
                    )
                    sq = pt.tile([128, 512], F16, name="sqk", tag="sq")
                    nc.scalar.activation(sq[:], ps[:], AF.Square)
                    nc.tensor.matmul(
                        ssqk_ps[:], cs["onesk"][:], sq[:],
                        start=(co == 0), stop=(co == 3),
                    )
                sd = prs.tile([1, 512], F32, name="sdk", tag="sdk")
                nc.scalar.activation(sd[:], ssqk_ps[:], AF.Sqrt, bias=cs["eps"][:])
                rs = prs.tile([1, 512], F32, name="rsk", tag="rsk")
                nc.vector.reciprocal(rs[:], sd[:])
                bck = prs.tile([128, 512], F32, name="bck", tag="bck")
                nc.gpsimd.partition_broadcast(bck[:], rs[:])
                nc.vector.tensor_mul(st["c2ks"][:, tsl], st["c2ks"][:, tsl], bck[:])
                nc.vector.tensor_mul(st["s2ks"][:, tsl], st["s2ks"][:, tsl], bck[:])
                for vt in range(4):
                    ps = pp.tile([128, 512], F32, name="psv", tag="ps")
                    for kc in range(16):
                        nc.tensor.matmul(
                            ps[:], xs[:, kc, vt * 128:(vt + 1) * 128], wvt[:, kc, :],
                            start=(kc == 0), stop=(kc == 15),
                        )
                    nc.scalar.activation(v_res[:, tk * 4 + vt, :], ps[:], AF.Copy)

    def p2_head(tc, cs, pools, g, hh, qR, kR, filler=None):
        """One head: both q-chunks' S matmuls + exps first (one long PE
        stretch; exp of chunk 0 overlaps S matmuls of chunk 1), then the
        DVE den trees, then yt/den matmuls and the division.  The den
        matmul sits AFTER yt so the in-order PE never waits on the tree.
        `filler` emits independent PE work (o-proj partials) between the
        S matmuls and the exp-dependent yt matmuls."""
        pS, p8, p4, p2t, p1t, prc, pbc, ppS, ppd, ppy = pools
        h = g * 4 + hh
        g128 = slice(g * 128, (g + 1) * 128)
        S_sbs, t1s = [], []
        for qc in range(2):
            qsl = slice(qc * 512, (qc + 1) * 512)
            S_sb = pS.tile([128, 16, 512], F16, name="S_sb", tag="S")
            for j in range(8):
                sps = ppS.tile([128, 2, 512], F32, name="sps", tag="sps")
                for i in range(2):
                    kc = 2 * j + i
                    nc.tensor.matmul(
                        sps[:, i, :], kR[:, kc * 128:(kc + 1) * 128], qR[:, qsl],
                        start=True, stop=True,
                    )
                nc.scalar.activation(S_sb[:, 2 * j:2 * j + 2, :], sps[:], AF.Exp)
            S_sbs.append(S_sb)
        if filler is not None:
            filler()
        for qc in range(2):
            # denominator: 4-level pairwise tree on DVE (fp16, 2x mode)
            S_sb = S_sbs[qc]
            t8 = p8.tile([128, 8, 512], F16, name="t8", tag="t8")
            nc.vector.tensor_add(t8[:], S_sb[:, 0:8, :], S_sb[:, 8:16, :])
            t4 = p4.tile([128, 4, 512], F16, name="t4", tag="t4")
            nc.vector.tensor_add(t4[:], t8[:, 0:4, :], t8[:, 4:8, :])
            t2 = p2t.tile([128, 2, 512], F16, name="t2", tag="t2")
            nc.vector.tensor_add(t2[:], t4[:, 0:2, :], t4[:, 2:4, :])
            t1 = p1t.tile([128, 512], F16, name="t1", tag="t1")
            nc.vector.tensor_add(t1[:], t2[:, 0, :], t2[:, 1, :])
            t1s.append(t1)
        for qc in range(2):
            qsl = slice(qc * 512, (qc + 1) * 512)
            yt_ps = ppy.tile([128, 512], F32, name="ytp", tag="ytp")
            for kc in range(16):
                nc.tensor.matmul(
                    yt_ps[:], st["v"][:, kc, g128], S_sbs[qc][:, kc, :],
                    start=(kc == 0), stop=(kc == 15),
                )
            den_ps = ppd.tile([1, 512], F32, name="den", tag="den")
            nc.tensor.matmul(den_ps[:], cs["ones"][:], t1s[qc][:],
                             start=True, stop=True)
            rcp = prc.tile([1, 512], F16, name="rcp", tag="rcp")
            with nc.allow_low_precision(reason="softmax denom fits fp16"):
                nc.vector.reciprocal(rcp[:], den_ps[:])
            bcr = pbc.tile([128, 512], F16, name="bcr", tag="bcr")
            nc.gpsimd.partition_broadcast(bcr[:], rcp[:])
            nc.vector.tensor_mul(st["yT"][:, h, qsl], yt_ps[:], bcr[:])

    def p2(tc, cs):
        """attention over 4 kv-groups x 4 heads x 2 q-chunks."""
        kT_res, yT_res = st["kT"], st["yT"]
        c2qs, s2qs, c2ks, s2ks = st["c2qs"], st["s2qs"], st["c2ks"], st["s2ks"]
        with tc.tile_pool(name="ksw", bufs=2) as pks, \
             tc.tile_pool(name="krp", bufs=2) as pkr, \
             tc.tile_pool(name="qh", bufs=2) as pqh, \
             tc.tile_pool(name="Sp", bufs=2) as pS, \
             tc.tile_pool(name="tr8", bufs=1) as p8, \
             tc.tile_pool(name="tr4", bufs=1) as p4, \
             tc.tile_pool(name="tr2", bufs=1) as p2t, \
             tc.tile_pool(name="tr1", bufs=2) as p1t, \
             tc.tile_pool(name="rcb", bufs=2) as prc, \
             tc.tile_pool(name="bcb", bufs=2) as pbc, \
             tc.tile_pool(name="woh", bufs=2) as pwh, \
             tc.tile_pool(name="sps", bufs=2, space="PSUM") as ppS, \
             tc.tile_pool(name="dnp", bufs=1, space="PSUM") as ppd, \
             tc.tile_pool(name="p3a", bufs=1, space="PSUM") as pp3a, \
             tc.tile_pool(name="ytp", bufs=2, space="PSUM") as ppy:
            pools = (pS, p8, p4, p2t, p1t, prc, pbc, ppS, ppd, ppy)
            woch = [None]

            def filler(h):
                # o-proj partial tiles (yc 0..7) as PE filler while the
                # ACT exp stream catches up.  4 of the 32 (co,qt) tiles
                # per head over heads 8..15.
                def emit():
                    co = (h - 8) // 2
                    if h % 2 == 0 and h < 16:
                        w = pwh.tile([128, 8, 512], F16, name="woch", tag="woch")
                        nc.sync.dma_start(
                            w[:], rp(wo)[:, 0:8, co * 512:(co + 1) * 512]
                        )
                        woch[0] = w
                    for i in range(4):
                        idx = (h - 8) * 4 + i
                        co, qt = idx // 8, idx % 8
                        ps3 = pp3a.tile([128, 512], F32, name="ps3", tag="p3")
                        for yc in range(8):
                            nc.tensor.matmul(
                                ps3[:],
                                st["yT"][:, yc, qt * 128:(qt + 1) * 128],
                                woch[0][:, yc, :],
                                start=(yc == 0), stop=(yc == 7),
                            )
                        nc.scalar.activation(st["part"][:, idx, :], ps3[:],
                                             AF.Copy)
                return emit

            def rope_q(h):
                # per-head rope chain; emitted one head AHEAD of its use so
                # the in-order DVE queue has qR ready before the S matmuls
                qTt = pqh.tile([128, TQ], F16, name="qTt", tag="qTt")
                nc.sync.dma_start(qTt[:], r3(qTs)[h])
                qSw = pqh.tile([128, TQ], F16, name="qSw", tag="qSw")
                nc.sync.dma_start(qSw[0:64, :], r3(qTs)[h, 64:128, :])
                nc.sync.dma_start(qSw[64:128, :], r3(qTs)[h, 0:64, :])
                qA = pqh.tile([128, TQ], F16, name="qA", tag="qA", bufs=1)
                nc.vector.tensor_mul(qA[:], qTt[:], c2qs[:])
                nc.vector.tensor_mul(qSw[:], qSw[:], s2qs[:])
                qR = pqh.tile([128, TQ], F16, name="qR", tag="qR")
                nc.vector.tensor_add(qR[:], qA[:], qSw[:])
                return qR

            qR_cur = rope_q(0)
            for g in range(N_KV_HEAD):
                kSw = pks.tile([128, T], F16, name="kSw", tag="kSw", bufs=1)
                nc.sync.dma_start(kSw[0:64, :], kT_res[64:128, g, :])
                nc.sync.dma_start(kSw[64:128, :], kT_res[0:64, g, :])
                kA = pkr.tile([128, T], F16, name="kA", tag="kA", bufs=1)
                nc.vector.tensor_mul(kA[:], kT_res[:, g, :], c2ks[:])
                nc.vector.tensor_mul(kSw[:], kSw[:], s2ks[:])
                kR = pkr.tile([128, T], F16, name="kR", tag="kR")
                nc.vector.tensor_add(kR[:], kA[:], kSw[:])
                for hh in range(4):
                    h = g * 4 + hh
                    qR_next = rope_q(h + 1) if h + 1 < N_HEAD else None
                    p2_head(tc, cs, pools, g, hh, qR_cur, kR,
                            filler=filler(h) if h >= 8 else None)
                    qR_cur = qR_next

    def p3(tc, cs):
        """output projection finish: yc 8..15 half + staged yc 0..7
        partials (computed during P2)."""
        yT_res = st["yT"]
        with tc.tile_pool(name="woc", bufs=2) as pwo, \
             tc.tile_pool(name="ev3", bufs=4) as pe3, \
             tc.tile_pool(name="pp3", bufs=4, space="PSUM") as pp3:
            for co in range(4):
                woc = pwo.tile([128, 8, 512], F16, name="woc", tag="woc")
                nc.sync.dma_start(
                    woc[:], rp(wo)[:, 8:16, co * 512:(co + 1) * 512]
                )
                for qt in range(8):
                    idx = co * 8 + qt
                    ps = pp3.tile([128, 512], F32, name="pso", tag="ps")
                    for yc in range(8):
                        nc.tensor.matmul(
                            ps[:], yT_res[:, 8 + yc, qt * 128:(qt + 1) * 128],
                            woc[:, yc, :],
                            start=(yc == 0), stop=(yc == 7),
                        )
                    osb = pe3.tile([128, 512], F32, name="osb", tag="osb")
                    nc.vector.tensor_add(osb[:], ps[:], st["part"][:, idx, :])
                    nc.gpsimd.dma_start(
                        out[qt * 128:(qt + 1) * 128, co * 512:(co + 1) * 512],
                        osb[:],
                    )

    with tile.TileContext(nc, trace_sim=trace_sim) as tc:
        with tc.tile_pool(name="const", bufs=1) as cpool:
            cs = {}
            for nm, src in (("ones", ones_d), ("onesq", onesq_d),
                            ("onesk", onesk_d)):
                t = cpool.tile([128, 1], F16, name=nm + "_t")
                nc.sync.dma_start(t[:], src)
                cs[nm] = t
            cs["eps"] = cpool.tile([1, 1], F32, name="eps_t")
            nc.sync.dma_start(cs["eps"][:], eps_d)
            cs["qnw"] = cpool.tile([128, 16], F32, name="qnw_t")
            nc.sync.dma_start(cs["qnw"][:], qnw)
            cs["knw"] = cpool.tile([128, 4], F32, name="knw_t")
            nc.sync.dma_start(cs["knw"][:], knw)

            for rep in range(reps):
                with tc.tile_pool(name="resid", bufs=1) as pres, \
                     tc.tile_pool(name="tabs", bufs=1) as ptab:
                    st.clear()
                    st["pres"], st["ptab"] = pres, ptab
                    with tc.tile_pool(name="wkv", bufs=1) as pwkv, \
                         tc.tile_pool(name="pp1", bufs=4, space="PSUM") as pp, \
                         tc.tile_pool(name="ssqp", bufs=1, space="PSUM") as pps:
                        st["pwkv"] = pwkv
                        st["pp"], st["pps"] = pp, pps
                        p1a(tc, cs)
                        p1b(tc, cs)
                    p2(tc, cs)
                    p3(tc, cs)

    nc.compile()
    return nc


def _make_in_maps(inputs):
    F16 = np.float16
    x = np.asarray(inputs["x"], np.float32)
    cos = np.asarray(inputs["cos"], np.float32)
    sin = np.asarray(inputs["sin"], np.float32)
    wq = np.ascontiguousarray(np.asarray(inputs["wq"], np.float32).astype(F16))
    wk = np.ascontiguousarray(np.asarray(inputs["wk"], np.float32).astype(F16))
    wv = np.ascontiguousarray(np.asarray(inputs["wv"], np.float32).astype(F16))
    wo = np.ascontiguousarray(np.asarray(inputs["wo"], np.float32).astype(F16))
    qnw = np.ascontiguousarray(
        np.asarray(inputs["q_norm_w"], np.float32).reshape(16, 128).T
    )
    knw = np.ascontiguousarray(
        np.asarray(inputs["k_norm_w"], np.float32).reshape(4, 128).T
    )

    cf = cos[0, :, 0, :].T  # (64, T)
    sf = sin[0, :, 0, :].T
    c2k = np.concatenate([cf, cf], 0)  # (128, T)
    s2k = np.concatenate([sf, -sf], 0)
    scale = 1.0 / np.sqrt(np.float32(HEAD_DIM))
    c2k16 = np.ascontiguousarray(c2k.astype(F16))
    s2k16 = np.ascontiguousarray(s2k.astype(F16))

    in_maps = []
    for c in range(N_CORES):
        b, r0 = c // 2, (c % 2) * TQ
        xT = np.ascontiguousarray(x[b].T.astype(F16))
        in_maps.append({
            "xT": xT,
            "xTq": np.ascontiguousarray(xT[:, r0:r0 + TQ]),
            "wq": wq, "wk": wk, "wv": wv, "wo": wo,
            "c2q": np.ascontiguousarray((c2k[:, r0:r0 + TQ] * scale).astype(F16)),
            "s2q": np.ascontiguousarray((s2k[:, r0:r0 + TQ] * scale).astype(F16)),
            "c2k": c2k16, "s2k": s2k16,
            "qnw": qnw, "knw": knw,
        })
    return in_maps


def run(inputs, **spmd_kwargs):
    from concourse import bass_utils

    if "nc" not in _CACHE:
        _CACHE["nc"] = _build_nc()
    nc = _CACHE["nc"]
    res = bass_utils.run_bass_kernel_spmd(
        nc, _make_in_maps(inputs), core_ids=list(range(N_CORES)), **spmd_kwargs
    )
    out = np.empty((B, T, C), np.float32)
    for c in range(N_CORES):
        b, r0 = c // 2, (c % 2) * TQ
        out[b, r0:r0 + TQ, :] = res.results[c]["out"]
    return out, res


def kernel(**inputs):
    out, _ = run(inputs)
    return out
